# revision 15
# baseline (speedup 1.0000x reference)
"""SSIM-based loss kernel for Trainium2 (8 NeuronCores, data-parallel over batch).

Computes: loss = 1 - (1 + mean(SSIM(sigmoid(seg), sigmoid(edge)))) / 2
for seg, edge of shape [32, 1, 512, 512] fp32, SSIM with a 7x7 gaussian
window (sigma=1.5), SAME zero-padding, C1=0.01^2, C2=0.03^2.

Sharding: batch dim across 8 cores (4 images each). Each core returns
per-partition partial sums of the ssim map; the host reduces and forms the
scalar loss.

v3 notes (HW-calibrated): DVE tensor_tensor bf16 SBUF runs 2x, STT runs 1x,
ACT is 1x with ~(fix+FD)/1.2GHz cost, PSUM reads are 1x. So the pointwise
chain is built from TT ops on bf16 with constants folded into ACT bias at
PSUM readout. Step-1/step-2 PSUM tiles are bank-paired so each readout
instruction covers two maps (FD=1024), halving fixed costs. The final
multiply+reduce is a fused STT with accum_out. den products run on the
otherwise-idle GPSIMD engine.

Math (per pixel, after 7x7 gaussian blur E[.]):
  pa = (mu1+mu2)/sqrt2, pb = (mu1-mu2)/sqrt2   [blur pipes of P=s+e, M=s-e]
  pu = E[s^2]+E[e^2]  (from (blur(P^2)+blur(M^2))/2)
  pv = 2 E[se]        (from (blur(P^2)-blur(M^2))/2)
  x = pa^2, y = pb^2;  w1 = x-y = 2 mu1 mu2;  w2 = x+y = mu1^2+mu2^2
  tv = pv + C2, tu = pu + C2
  gamma = tv - w1 (= 2 sigma12 + C2),  delta = tu - w2 (= sig1^2+sig2^2+C2)
  num = (w1+C1)*gamma,  den = (w2+C1)*delta,  ssim = num/den
"""

import numpy as np
import ml_dtypes

import concourse.bass as bass
import concourse.bacc as bacc
import concourse.tile as tile
import concourse.mybir as mybir
from concourse.bass_utils import run_bass_kernel_spmd

WS = 7
HW = WS // 2
SIGMA = 1.5
C1 = 0.01 ** 2
C2 = 0.03 ** 2

N_CORES = 8
IMG = 512
P = 128
PER_CORE = 4

# halo chunking: out regions [O[c], O[c+1]), input rows [R[c], R[c]+128)
O = [0, 122, 244, 366, 488, 512]
R = [0, 119, 241, 363, 384]
NC5 = 5

F32 = mybir.dt.float32
BF16 = mybir.dt.bfloat16
AF = mybir.ActivationFunctionType
OP = mybir.AluOpType
BF = ml_dtypes.bfloat16

GP_DEN = False  # GPSIMD float TT unsupported on HW (integer/power only)


def _gauss():
    x = np.arange(WS, dtype=np.float64)
    g = np.exp(-((x - HW) ** 2) / (2.0 * SIGMA ** 2))
    return g / g.sum()


def _band_tiles(scale):
    g = _gauss() * scale
    tiles = []
    for c in range(NC5):
        w = O[c + 1] - O[c]
        t = np.zeros((P, w), dtype=np.float64)
        for r in range(P):
            i = R[c] + r
            for j in range(w):
                d = (O[c] + j) - i
                if -HW <= d <= HW:
                    t[r, j] = g[d + HW]
        tiles.append(t.astype(np.float32))
    return tiles


_CACHE = {}


def _build():
    if "nc" in _CACHE:
        return _CACHE["nc"]

    nc = bacc.Bacc(None)

    seg_d = nc.dram_tensor("seg", [PER_CORE, IMG, IMG], F32, kind="ExternalInput")
    edge_d = nc.dram_tensor("edge", [PER_CORE, IMG, IMG], F32, kind="ExternalInput")
    out_d = nc.dram_tensor("out", [P, 1], F32, kind="ExternalOutput")

    # Band variants: 0: step1 (scale 1); 1: mu pipes (1/sqrt2); 2: +1/2; 3: -1/2
    variants = [1.0, 1.0 / np.sqrt(2.0), 0.5, -0.5]
    packed, offsets = [], []
    col = 0
    for v in variants:
        offs = []
        for t in _band_tiles(v):
            offs.append((col, t.shape[1]))
            packed.append(t)
            col += t.shape[1]
        offsets.append(offs)
    band_np = np.concatenate(packed, axis=1).astype(BF)  # [128, 2048] bf16
    band_d = nc.inline_tensor(band_np, name="band")

    # chunk pairs for FD=1024 ops: (0,1), (2,3), (4,)
    PAIRS = [(0, 2), (2, 2), (4, 1)]

    with tile.TileContext(nc) as tc:
        with (
            tc.tile_pool(name="const", bufs=1) as constp,
            tc.tile_pool(name="io", bufs=2) as iop,
            tc.tile_pool(name="sig", bufs=2) as sigp,
            tc.tile_pool(name="maps", bufs=2) as mapp,
            tc.tile_pool(name="zmaps", bufs=2) as zp,
            tc.tile_pool(name="ro", bufs=2) as rop,
            tc.tile_pool(name="chain", bufs=2) as chp,
            tc.tile_pool(name="acc", bufs=1) as accp,
            tc.tile_pool(name="psz", bufs=1, space="PSUM") as psz,
            tc.tile_pool(name="ps2", bufs=1, space="PSUM") as ps2,
        ):
            band = constp.tile([P, band_np.shape[1]], BF16)
            nc.sync.dma_start(band[:], band_d[:])

            def band_ap(v, c):
                c0, w = offsets[v][c]
                return band[:, c0:c0 + w], w

            partials = accp.tile([P, PER_CORE * 3], F32)
            nc.vector.memset(partials[:], 0.0)
            c2c = constp.tile([P, 1], F32)
            nc.vector.memset(c2c[:], C2)

            def load_and_premaps(b):
                # DMA (split across Sync and GpSimd queues) + one FD<=2048
                # Sigmoid per chunk-pair covering both tensors.
                set_t = sigp.tile([P, 2, NC5, IMG], BF16, tag="set")
                for c0, w in PAIRS:
                    raw = iop.tile([P, 2, 2, IMG], F32, tag="raw")
                    for i in range(w):
                        c = c0 + i
                        nc.sync.dma_start(raw[:, 0, i, :], seg_d[b, R[c]:R[c] + P, :])
                        nc.gpsimd.dma_start(raw[:, 1, i, :], edge_d[b, R[c]:R[c] + P, :])
                    nc.scalar.activation(set_t[:, :, c0:c0 + w, :], raw[:, :, :w, :], AF.Sigmoid)

                sf = set_t[:, 0, :, :].rearrange("p c w -> p (c w)")
                ef = set_t[:, 1, :, :].rearrange("p c w -> p (c w)")
                Pt = mapp.tile([P, NC5, IMG], BF16, tag="P")
                Mt = mapp.tile([P, NC5, IMG], BF16, tag="M")
                nc.vector.tensor_tensor(Pt[:].rearrange("p c w -> p (c w)"), sf, ef, OP.add)
                nc.vector.tensor_tensor(Mt[:].rearrange("p c w -> p (c w)"), sf, ef, OP.subtract)
                P2t = mapp.tile([P, NC5, IMG], BF16, tag="P2")
                M2t = mapp.tile([P, NC5, IMG], BF16, tag="M2")
                nc.scalar.activation(P2t[:], Pt[:], AF.Square)
                nc.scalar.activation(M2t[:], Mt[:], AF.Square)
                return Pt, Mt, P2t, M2t

            def step1(maps, zmaps, k):
                # blur step 1: z[w, j] (transposed, halo layout along w).
                # PSUM bank-paired: [zP|zM] copied by ACT, [zP2|zM2] by DVE.
                Pt, Mt, P2t, M2t = maps
                zPM, z22 = zmaps
                pzPM = psz.tile([P, 2, IMG], F32, tag="pzPM")
                pz22 = psz.tile([P, 2, IMG], F32, tag="pz22")
                for half, src in ((0, Pt), (1, Mt)):
                    for c in range(NC5):
                        rhs, w = band_ap(0, c)
                        nc.tensor.matmul(
                            pzPM[:, half, O[c]:O[c + 1]],
                            src[:, c, R[k]:R[k] + P], rhs,
                            start=(c == 0), stop=(c == NC5 - 1))
                for half, src in ((0, P2t), (1, M2t)):
                    for c in range(NC5):
                        rhs, w = band_ap(0, c)
                        nc.tensor.matmul(
                            pz22[:, half, O[c]:O[c + 1]],
                            src[:, c, R[k]:R[k] + P], rhs,
                            start=(c == 0), stop=(c == NC5 - 1))
                nc.scalar.copy(zPM[:, k, :, :], pzPM[:])
                nc.vector.tensor_copy(z22[:, k, :, :], pz22[:])

            def step2(zmaps, ro, k):
                # blur step 2 (bank-paired [pa|pb], [pu|pv]) + PSUM readout
                zPM, z22 = zmaps
                xy, tuv = ro
                wk = O[k + 1] - O[k]
                pab = ps2.tile([P, 2, IMG], F32, tag="pab")
                puv = ps2.tile([P, 2, IMG], F32, tag="puv")
                bmu, _ = band_ap(1, k)
                bph, _ = band_ap(2, k)
                bnh, _ = band_ap(3, k)
                nc.tensor.matmul(pab[:wk, 0, :], bmu, zPM[:, k, 0, :], start=True, stop=True)
                nc.tensor.matmul(pab[:wk, 1, :], bmu, zPM[:, k, 1, :], start=True, stop=True)
                nc.tensor.matmul(puv[:wk, 0, :], bph, z22[:, k, 0, :], start=True, stop=False)
                nc.tensor.matmul(puv[:wk, 0, :], bph, z22[:, k, 1, :], start=False, stop=True)
                nc.tensor.matmul(puv[:wk, 1, :], bph, z22[:, k, 0, :], start=True, stop=False)
                nc.tensor.matmul(puv[:wk, 1, :], bnh, z22[:, k, 1, :], start=False, stop=True)
                # x,y = pa^2,pb^2 ; tu,tv = pu,pv + C2  (FD=1024 each)
                nc.scalar.activation(xy[:wk, k, :, :], pab[:wk, :, :], AF.Square)
                nc.scalar.activation(tuv[:wk, k, :, :], puv[:wk, :, :], AF.Identity, bias=c2c[:wk, :])

            def chain(ro, b, pi):
                # pointwise chain, TT-heavy (bf16 2x), per chunk-pair so it
                # pipelines with step-2. Garbage partition rows (wk..128)
                # never reach the reduction.
                xy, tuv = ro
                c0, w = PAIRS[pi]
                wk = O[c0 + 1] - O[c0]
                xs = xy[:, c0:c0 + w, 0, :]
                ys = xy[:, c0:c0 + w, 1, :]
                tus = tuv[:, c0:c0 + w, 0, :]
                tvs = tuv[:, c0:c0 + w, 1, :]
                w1 = chp.tile([P, 2, IMG], BF16, tag="w1")
                w2 = chp.tile([P, 2, IMG], BF16, tag="w2")
                nc.vector.tensor_tensor(w1[:, :w, :], xs, ys, OP.subtract)
                nc.vector.tensor_tensor(w2[:, :w, :], xs, ys, OP.add)
                ga = chp.tile([P, 2, IMG], BF16, tag="ga")
                de = chp.tile([P, 2, IMG], BF16, tag="de")
                nc.vector.tensor_tensor(ga[:, :w, :], tvs, w1[:, :w, :], OP.subtract)
                nc.vector.tensor_tensor(de[:, :w, :], tus, w2[:, :w, :], OP.subtract)
                # num = (w1+C1)*gamma (STT 1x); den = (w2+C1)*delta
                nu = chp.tile([P, 2, IMG], BF16, tag="nu")
                dn = chp.tile([P, 2, IMG], F32, tag="dn")
                nc.vector.scalar_tensor_tensor(
                    nu[:, :w, :], w1[:, :w, :], C1, ga[:, :w, :], OP.add, OP.mult)
                nc.vector.scalar_tensor_tensor(
                    dn[:, :w, :], w2[:, :w, :], C1, de[:, :w, :], OP.add, OP.mult)
                rc = chp.tile([P, 2, IMG], F32, tag="rc")
                nc.vector.reciprocal_approx_fast(rc[:, :w, :], dn[:, :w, :])
                # fused ssim = num*rc with accumulation
                jk = chp.tile([P, 2, IMG], BF16, tag="ga")
                nc.vector.scalar_tensor_tensor(
                    jk[:wk, :w, :], nu[:wk, :w, :], 1.0,
                    rc[:wk, :w, :], OP.mult, OP.mult,
                    accum_out=partials[:wk, b * 3 + pi: b * 3 + pi + 1],
                )

            # Two images interleaved through the k-loop: image b1's step-1
            # overlaps b0's copy/step-2/readout tail on the shared PSUM
            # banks, and vice versa.
            for g in range(PER_CORE // 2):
                b0, b1 = 2 * g, 2 * g + 1
                maps0 = load_and_premaps(b0)
                maps1 = load_and_premaps(b1)
                zm0 = (zp.tile([P, NC5, 2, IMG], BF16, tag="zPM", name="zPM0"),
                       zp.tile([P, NC5, 2, IMG], BF16, tag="z22", name="z220"))
                zm1 = (zp.tile([P, NC5, 2, IMG], BF16, tag="zPM", name="zPM1"),
                       zp.tile([P, NC5, 2, IMG], BF16, tag="z22", name="z221"))
                ro0 = (rop.tile([P, NC5, 2, IMG], BF16, tag="xy", name="xy0"),
                       rop.tile([P, NC5, 2, IMG], BF16, tag="tuv", name="tuv0"))
                ro1 = (rop.tile([P, NC5, 2, IMG], BF16, tag="xy", name="xy1"),
                       rop.tile([P, NC5, 2, IMG], BF16, tag="tuv", name="tuv1"))
                for k in range(NC5):
                    step1(maps0, zm0, k)
                    step1(maps1, zm1, k)
                    step2(zm0, ro0, k)
                    step2(zm1, ro1, k)
                    # chain pair p is ready once chunks c0..c0+w done
                    for pi, (c0, w) in enumerate(PAIRS):
                        if c0 + w - 1 == k:
                            chain(ro0, b0, pi)
                            chain(ro1, b1, pi)

            final = accp.tile([P, 1], F32)
            nc.vector.tensor_reduce(final[:], partials[:], mybir.AxisListType.X, OP.add)
            nc.sync.dma_start(out_d[:], final[:])

    nc.compile()
    _CACHE["nc"] = nc
    return nc


def kernel(seg: np.ndarray, edge: np.ndarray) -> np.ndarray:
    nc = _build()
    seg = np.ascontiguousarray(seg, dtype=np.float32).reshape(N_CORES, PER_CORE, IMG, IMG)
    edge = np.ascontiguousarray(edge, dtype=np.float32).reshape(N_CORES, PER_CORE, IMG, IMG)
    in_maps = [{"seg": seg[c], "edge": edge[c]} for c in range(N_CORES)]
    res = run_bass_kernel_spmd(nc, in_maps, list(range(N_CORES)))
    total = 0.0
    for c in range(N_CORES):
        total += float(res.results[c]["out"].astype(np.float64).sum())
    mssim = total / (32.0 * IMG * IMG)
    return np.float32(1.0 - (1.0 + mssim) / 2.0)


# revision 16
# speedup vs baseline: 1.0615x; 1.0615x over previous
"""SSIM-based loss kernel for Trainium2 (8 NeuronCores, data-parallel over batch).

Computes: loss = 1 - (1 + mean(SSIM(sigmoid(seg), sigmoid(edge)))) / 2
for seg, edge of shape [32, 1, 512, 512] fp32, SSIM with a 7x7 gaussian
window (sigma=1.5), SAME zero-padding, C1=0.01^2, C2=0.03^2.

Sharding: batch dim across 8 cores (4 images each). Each core returns
per-partition partial sums of the ssim map; the host reduces and forms the
scalar loss.

v3 notes (HW-calibrated): DVE tensor_tensor bf16 SBUF runs 2x, STT runs 1x,
ACT is 1x with ~(fix+FD)/1.2GHz cost, PSUM reads are 1x. So the pointwise
chain is built from TT ops on bf16 with constants folded into ACT bias at
PSUM readout. Step-1/step-2 PSUM tiles are bank-paired so each readout
instruction covers two maps (FD=1024), halving fixed costs. The final
multiply+reduce is a fused STT with accum_out. den products run on the
otherwise-idle GPSIMD engine.

Math (per pixel, after 7x7 gaussian blur E[.]):
  pa = (mu1+mu2)/sqrt2, pb = (mu1-mu2)/sqrt2   [blur pipes of P=s+e, M=s-e]
  pu = E[s^2]+E[e^2]  (from (blur(P^2)+blur(M^2))/2)
  pv = 2 E[se]        (from (blur(P^2)-blur(M^2))/2)
  x = pa^2, y = pb^2;  w1 = x-y = 2 mu1 mu2;  w2 = x+y = mu1^2+mu2^2
  tv = pv + C2, tu = pu + C2
  gamma = tv - w1 (= 2 sigma12 + C2),  delta = tu - w2 (= sig1^2+sig2^2+C2)
  num = (w1+C1)*gamma,  den = (w2+C1)*delta,  ssim = num/den
"""

import numpy as np
import ml_dtypes

import concourse.bass as bass
import concourse.bacc as bacc
import concourse.tile as tile
import concourse.mybir as mybir
from concourse.bass_utils import run_bass_kernel_spmd

WS = 7
HW = WS // 2
SIGMA = 1.5
C1 = 0.01 ** 2
C2 = 0.03 ** 2

N_CORES = 8
IMG = 512
P = 128
PER_CORE = 4

# halo chunking: out regions [O[c], O[c+1]), input rows [R[c], R[c]+128)
O = [0, 122, 244, 366, 488, 512]
R = [0, 119, 241, 363, 384]
NC5 = 5

F32 = mybir.dt.float32
BF16 = mybir.dt.bfloat16
AF = mybir.ActivationFunctionType
OP = mybir.AluOpType
BF = ml_dtypes.bfloat16

GP_DEN = False  # GPSIMD float TT unsupported on HW (integer/power only)


def _gauss():
    x = np.arange(WS, dtype=np.float64)
    g = np.exp(-((x - HW) ** 2) / (2.0 * SIGMA ** 2))
    return g / g.sum()


def _band_tiles(scale):
    g = _gauss() * scale
    tiles = []
    for c in range(NC5):
        w = O[c + 1] - O[c]
        t = np.zeros((P, w), dtype=np.float64)
        for r in range(P):
            i = R[c] + r
            for j in range(w):
                d = (O[c] + j) - i
                if -HW <= d <= HW:
                    t[r, j] = g[d + HW]
        tiles.append(t.astype(np.float32))
    return tiles


_CACHE = {}


def _build():
    if "nc" in _CACHE:
        return _CACHE["nc"]

    nc = bacc.Bacc(None)

    seg_d = nc.dram_tensor("seg", [PER_CORE, IMG, IMG], F32, kind="ExternalInput")
    edge_d = nc.dram_tensor("edge", [PER_CORE, IMG, IMG], F32, kind="ExternalInput")
    out_d = nc.dram_tensor("out", [P, 1], F32, kind="ExternalOutput")

    # Band variants: 0: step1 (scale 1); 1: mu pipes (1/sqrt2); 2: +1/2; 3: -1/2
    variants = [1.0, 1.0 / np.sqrt(2.0), 0.5, -0.5]
    packed, offsets = [], []
    col = 0
    for v in variants:
        offs = []
        for t in _band_tiles(v):
            offs.append((col, t.shape[1]))
            packed.append(t)
            col += t.shape[1]
        offsets.append(offs)
    band_np = np.concatenate(packed, axis=1).astype(BF)  # [128, 2048] bf16
    band_d = nc.inline_tensor(band_np, name="band")

    # chunk pairs for FD=1024 ops: (0,1), (2,3), (4,)
    PAIRS = [(0, 2), (2, 2), (4, 1)]

    with tile.TileContext(nc) as tc:
        with (
            tc.tile_pool(name="const", bufs=1) as constp,
            tc.tile_pool(name="io", bufs=2) as iop,
            tc.tile_pool(name="sig", bufs=2) as sigp,
            tc.tile_pool(name="maps", bufs=2) as mapp,
            tc.tile_pool(name="zmaps", bufs=2) as zp,
            tc.tile_pool(name="ro", bufs=2) as rop,
            tc.tile_pool(name="chain", bufs=2) as chp,
            tc.tile_pool(name="acc", bufs=1) as accp,
            tc.tile_pool(name="psz", bufs=1, space="PSUM") as psz,
            tc.tile_pool(name="ps2", bufs=1, space="PSUM") as ps2,
        ):
            band = constp.tile([P, band_np.shape[1]], BF16)
            nc.sync.dma_start(band[:], band_d[:])

            def band_ap(v, c):
                c0, w = offsets[v][c]
                return band[:, c0:c0 + w], w

            partials = accp.tile([P, PER_CORE * 3], F32)
            nc.vector.memset(partials[:], 0.0)
            c2c = constp.tile([P, 1], F32)
            nc.vector.memset(c2c[:], C2)

            def load_and_premaps(b):
                # DMA (split across Sync and GpSimd queues) + one FD<=2048
                # Sigmoid per chunk-pair covering both tensors.
                set_t = sigp.tile([P, 2, NC5, IMG], BF16, tag="set")
                for c0, w in PAIRS:
                    raw = iop.tile([P, 2, 2, IMG], F32, tag="raw")
                    for i in range(w):
                        c = c0 + i
                        nc.sync.dma_start(raw[:, 0, i, :], seg_d[b, R[c]:R[c] + P, :])
                        nc.gpsimd.dma_start(raw[:, 1, i, :], edge_d[b, R[c]:R[c] + P, :])
                    nc.scalar.activation(set_t[:, :, c0:c0 + w, :], raw[:, :, :w, :], AF.Sigmoid)

                sf = set_t[:, 0, :, :].rearrange("p c w -> p (c w)")
                ef = set_t[:, 1, :, :].rearrange("p c w -> p (c w)")
                Pt = mapp.tile([P, NC5, IMG], BF16, tag="P")
                Mt = mapp.tile([P, NC5, IMG], BF16, tag="M")
                nc.vector.tensor_tensor(Pt[:].rearrange("p c w -> p (c w)"), sf, ef, OP.add)
                nc.vector.tensor_tensor(Mt[:].rearrange("p c w -> p (c w)"), sf, ef, OP.subtract)
                P2t = mapp.tile([P, NC5, IMG], BF16, tag="P2")
                M2t = mapp.tile([P, NC5, IMG], BF16, tag="M2")
                nc.scalar.activation(P2t[:], Pt[:], AF.Square)
                nc.scalar.activation(M2t[:], Mt[:], AF.Square)
                return Pt, Mt, P2t, M2t

            def step1(maps, zmaps, k):
                # blur step 1: z[w, j] (transposed, halo layout along w).
                # PSUM bank-paired: [zP|zM] copied by ACT, [zP2|zM2] by DVE.
                Pt, Mt, P2t, M2t = maps
                zPM, z22 = zmaps
                pzPM = psz.tile([P, 2, IMG], F32, tag="pzPM")
                pz22 = psz.tile([P, 2, IMG], F32, tag="pz22")
                for half, src in ((0, Pt), (1, Mt)):
                    for c in range(NC5):
                        rhs, w = band_ap(0, c)
                        nc.tensor.matmul(
                            pzPM[:, half, O[c]:O[c + 1]],
                            src[:, c, R[k]:R[k] + P], rhs,
                            start=(c == 0), stop=(c == NC5 - 1))
                for half, src in ((0, P2t), (1, M2t)):
                    for c in range(NC5):
                        rhs, w = band_ap(0, c)
                        nc.tensor.matmul(
                            pz22[:, half, O[c]:O[c + 1]],
                            src[:, c, R[k]:R[k] + P], rhs,
                            start=(c == 0), stop=(c == NC5 - 1))
                nc.scalar.copy(zPM[:, k, :, :], pzPM[:])
                nc.vector.tensor_copy(z22[:, k, :, :], pz22[:])

            def step2(zmaps, ro, k):
                # blur step 2 (bank-paired [pa|pb], [pu|pv]) + PSUM readout
                zPM, z22 = zmaps
                xy, tuv = ro
                wk = O[k + 1] - O[k]
                pab = ps2.tile([P, 2, IMG], F32, tag="pab")
                puv = ps2.tile([P, 2, IMG], F32, tag="puv")
                bmu, _ = band_ap(1, k)
                bph, _ = band_ap(2, k)
                bnh, _ = band_ap(3, k)
                nc.tensor.matmul(pab[:wk, 0, :], bmu, zPM[:, k, 0, :], start=True, stop=True)
                nc.tensor.matmul(pab[:wk, 1, :], bmu, zPM[:, k, 1, :], start=True, stop=True)
                nc.tensor.matmul(puv[:wk, 0, :], bph, z22[:, k, 0, :], start=True, stop=False)
                nc.tensor.matmul(puv[:wk, 0, :], bph, z22[:, k, 1, :], start=False, stop=True)
                nc.tensor.matmul(puv[:wk, 1, :], bph, z22[:, k, 0, :], start=True, stop=False)
                nc.tensor.matmul(puv[:wk, 1, :], bnh, z22[:, k, 1, :], start=False, stop=True)
                # x,y = pa^2,pb^2 ; tu,tv = pu,pv + C2  (FD=1024 each)
                nc.scalar.activation(xy[:wk, k, :, :], pab[:wk, :, :], AF.Square)
                nc.scalar.activation(tuv[:wk, k, :, :], puv[:wk, :, :], AF.Identity, bias=c2c[:wk, :])

            def chain(ro, b, pi):
                # pointwise chain, TT-heavy (bf16 2x), per chunk-pair so it
                # pipelines with step-2. Garbage partition rows (wk..128)
                # never reach the reduction.
                xy, tuv = ro
                c0, w = PAIRS[pi]
                wk = O[c0 + 1] - O[c0]
                xs = xy[:, c0:c0 + w, 0, :]
                ys = xy[:, c0:c0 + w, 1, :]
                tus = tuv[:, c0:c0 + w, 0, :]
                tvs = tuv[:, c0:c0 + w, 1, :]
                w1 = chp.tile([P, 2, IMG], BF16, tag="w1")
                w2 = chp.tile([P, 2, IMG], BF16, tag="w2")
                nc.vector.tensor_tensor(w1[:, :w, :], xs, ys, OP.subtract)
                nc.vector.tensor_tensor(w2[:, :w, :], xs, ys, OP.add)
                ga = chp.tile([P, 2, IMG], BF16, tag="ga")
                de = chp.tile([P, 2, IMG], BF16, tag="de")
                nc.vector.tensor_tensor(ga[:, :w, :], tvs, w1[:, :w, :], OP.subtract)
                nc.vector.tensor_tensor(de[:, :w, :], tus, w2[:, :w, :], OP.subtract)
                # num = (w1+C1)*gamma (STT 1x); den = (w2+C1)*delta
                nu = chp.tile([P, 2, IMG], BF16, tag="nu")
                dn = chp.tile([P, 2, IMG], F32, tag="dn")
                nc.vector.scalar_tensor_tensor(
                    nu[:, :w, :], w1[:, :w, :], C1, ga[:, :w, :], OP.add, OP.mult)
                nc.vector.scalar_tensor_tensor(
                    dn[:, :w, :], w2[:, :w, :], C1, de[:, :w, :], OP.add, OP.mult)
                rc = chp.tile([P, 2, IMG], F32, tag="rc")
                nc.vector.reciprocal_approx_fast(rc[:, :w, :], dn[:, :w, :])
                # fused ssim = num*rc with accumulation
                jk = chp.tile([P, 2, IMG], BF16, tag="ga")
                nc.vector.scalar_tensor_tensor(
                    jk[:wk, :w, :], nu[:wk, :w, :], 1.0,
                    rc[:wk, :w, :], OP.mult, OP.mult,
                    accum_out=partials[:wk, b * 3 + pi: b * 3 + pi + 1],
                )

            # Per-image streaming; k-interleaved emission so engine priority
            # order matches the pipeline (copy(k), readout(k), copy(k+1)...).
            for b in range(PER_CORE):
                maps = load_and_premaps(b)
                zm = (zp.tile([P, NC5, 2, IMG], BF16, tag="zPM", name="zPMt"),
                      zp.tile([P, NC5, 2, IMG], BF16, tag="z22", name="z22t"))
                ro = (rop.tile([P, NC5, 2, IMG], BF16, tag="xy", name="xyt"),
                      rop.tile([P, NC5, 2, IMG], BF16, tag="tuv", name="tuvt"))
                for k in range(NC5):
                    step1(maps, zm, k)
                    step2(zm, ro, k)
                    # chain pair p is ready once chunks c0..c0+w done
                    for pi, (c0, w) in enumerate(PAIRS):
                        if c0 + w - 1 == k:
                            chain(ro, b, pi)

            final = accp.tile([P, 1], F32)
            nc.vector.tensor_reduce(final[:], partials[:], mybir.AxisListType.X, OP.add)
            nc.sync.dma_start(out_d[:], final[:])

    nc.compile()
    _CACHE["nc"] = nc
    return nc


def kernel(seg: np.ndarray, edge: np.ndarray) -> np.ndarray:
    nc = _build()
    seg = np.ascontiguousarray(seg, dtype=np.float32).reshape(N_CORES, PER_CORE, IMG, IMG)
    edge = np.ascontiguousarray(edge, dtype=np.float32).reshape(N_CORES, PER_CORE, IMG, IMG)
    in_maps = [{"seg": seg[c], "edge": edge[c]} for c in range(N_CORES)]
    res = run_bass_kernel_spmd(nc, in_maps, list(range(N_CORES)))
    total = 0.0
    for c in range(N_CORES):
        total += float(res.results[c]["out"].astype(np.float64).sum())
    mssim = total / (32.0 * IMG * IMG)
    return np.float32(1.0 - (1.0 + mssim) / 2.0)


# revision 18
# speedup vs baseline: 1.0819x; 1.0193x over previous
"""SSIM-based loss kernel for Trainium2 (8 NeuronCores, data-parallel over batch).

Computes: loss = 1 - (1 + mean(SSIM(sigmoid(seg), sigmoid(edge)))) / 2
for seg, edge of shape [32, 1, 512, 512] fp32, SSIM with a 7x7 gaussian
window (sigma=1.5), SAME zero-padding, C1=0.01^2, C2=0.03^2.

Sharding: batch dim across 8 cores (4 images each). Each core returns
per-partition partial sums of the ssim map; the host reduces and forms the
scalar loss.

v3 notes (HW-calibrated): DVE tensor_tensor bf16 SBUF runs 2x, STT runs 1x,
ACT is 1x with ~(fix+FD)/1.2GHz cost, PSUM reads are 1x. So the pointwise
chain is built from TT ops on bf16 with constants folded into ACT bias at
PSUM readout. Step-1/step-2 PSUM tiles are bank-paired so each readout
instruction covers two maps (FD=1024), halving fixed costs. The final
multiply+reduce is a fused STT with accum_out. den products run on the
otherwise-idle GPSIMD engine.

Math (per pixel, after 7x7 gaussian blur E[.]):
  pa = (mu1+mu2)/sqrt2, pb = (mu1-mu2)/sqrt2   [blur pipes of P=s+e, M=s-e]
  pu = E[s^2]+E[e^2]  (from (blur(P^2)+blur(M^2))/2)
  pv = 2 E[se]        (from (blur(P^2)-blur(M^2))/2)
  x = pa^2, y = pb^2;  w1 = x-y = 2 mu1 mu2;  w2 = x+y = mu1^2+mu2^2
  tv = pv + C2, tu = pu + C2
  gamma = tv - w1 (= 2 sigma12 + C2),  delta = tu - w2 (= sig1^2+sig2^2+C2)
  num = (w1+C1)*gamma,  den = (w2+C1)*delta,  ssim = num/den
"""

import numpy as np
import ml_dtypes

import concourse.bass as bass
import concourse.bacc as bacc
import concourse.tile as tile
import concourse.mybir as mybir
from concourse.bass_utils import run_bass_kernel_spmd

WS = 7
HW = WS // 2
SIGMA = 1.5
C1 = 0.01 ** 2
C2 = 0.03 ** 2

N_CORES = 8
IMG = 512
P = 128
PER_CORE = 4

# halo chunking: out regions [O[c], O[c+1]), input rows [R[c], R[c]+128)
O = [0, 122, 244, 366, 488, 512]
R = [0, 119, 241, 363, 384]
NC5 = 5

F32 = mybir.dt.float32
BF16 = mybir.dt.bfloat16
AF = mybir.ActivationFunctionType
OP = mybir.AluOpType
BF = ml_dtypes.bfloat16

GP_DEN = False  # GPSIMD float TT unsupported on HW (integer/power only)


def _gauss():
    x = np.arange(WS, dtype=np.float64)
    g = np.exp(-((x - HW) ** 2) / (2.0 * SIGMA ** 2))
    return g / g.sum()


def _band_tiles(scale):
    g = _gauss() * scale
    tiles = []
    for c in range(NC5):
        w = O[c + 1] - O[c]
        t = np.zeros((P, w), dtype=np.float64)
        for r in range(P):
            i = R[c] + r
            for j in range(w):
                d = (O[c] + j) - i
                if -HW <= d <= HW:
                    t[r, j] = g[d + HW]
        tiles.append(t.astype(np.float32))
    return tiles


_CACHE = {}


def _build():
    if "nc" in _CACHE:
        return _CACHE["nc"]

    nc = bacc.Bacc(None)

    seg_d = nc.dram_tensor("seg", [PER_CORE, IMG, IMG], F32, kind="ExternalInput")
    edge_d = nc.dram_tensor("edge", [PER_CORE, IMG, IMG], F32, kind="ExternalInput")
    out_d = nc.dram_tensor("out", [P, 1], F32, kind="ExternalOutput")

    # Band variants: 0: step1 (scale 1); 1: mu pipes (1/sqrt2); 2: +1/2; 3: -1/2
    variants = [1.0, 1.0 / np.sqrt(2.0), 0.5, -0.5]
    packed, offsets = [], []
    col = 0
    for v in variants:
        offs = []
        for t in _band_tiles(v):
            offs.append((col, t.shape[1]))
            packed.append(t)
            col += t.shape[1]
        offsets.append(offs)
    band_np = np.concatenate(packed, axis=1).astype(BF)  # [128, 2048] bf16
    band_d = nc.inline_tensor(band_np, name="band")

    # chunk pairs for FD=1024 ops: (0,1), (2,3), (4,)
    PAIRS = [(0, 2), (2, 2), (4, 1)]

    with tile.TileContext(nc) as tc:
        with (
            tc.tile_pool(name="const", bufs=1) as constp,
            tc.tile_pool(name="io", bufs=3) as iop,
            tc.tile_pool(name="sig", bufs=2) as sigp,
            tc.tile_pool(name="maps", bufs=2) as mapp,
            tc.tile_pool(name="zmaps", bufs=2) as zp,
            tc.tile_pool(name="ro", bufs=2) as rop,
            tc.tile_pool(name="chain", bufs=2) as chp,
            tc.tile_pool(name="acc", bufs=1) as accp,
            tc.tile_pool(name="psz", bufs=1, space="PSUM") as psz,
            tc.tile_pool(name="ps2", bufs=1, space="PSUM") as ps2,
        ):
            band = constp.tile([P, band_np.shape[1]], BF16)
            nc.sync.dma_start(band[:], band_d[:])

            def band_ap(v, c):
                c0, w = offsets[v][c]
                return band[:, c0:c0 + w], w

            partials = accp.tile([P, PER_CORE * 3], F32)
            nc.vector.memset(partials[:], 0.0)
            c2c = constp.tile([P, 1], F32)
            nc.vector.memset(c2c[:], C2)

            def load_and_premaps(b):
                # DMA (split across Sync and GpSimd queues) + one FD<=2048
                # Sigmoid per chunk-pair covering both tensors.
                set_t = sigp.tile([P, 2, NC5, IMG], BF16, tag="set")
                for c0, w in PAIRS:
                    raw = iop.tile([P, 2, 2, IMG], F32, tag="raw")
                    for i in range(w):
                        c = c0 + i
                        nc.sync.dma_start(raw[:, 0, i, :], seg_d[b, R[c]:R[c] + P, :])
                        nc.gpsimd.dma_start(raw[:, 1, i, :], edge_d[b, R[c]:R[c] + P, :])
                    nc.scalar.activation(set_t[:, :, c0:c0 + w, :], raw[:, :, :w, :], AF.Sigmoid)

                sf = set_t[:, 0, :, :].rearrange("p c w -> p (c w)")
                ef = set_t[:, 1, :, :].rearrange("p c w -> p (c w)")
                Pt = mapp.tile([P, NC5, IMG], BF16, tag="P")
                Mt = mapp.tile([P, NC5, IMG], BF16, tag="M")
                nc.vector.tensor_tensor(Pt[:].rearrange("p c w -> p (c w)"), sf, ef, OP.add)
                nc.vector.tensor_tensor(Mt[:].rearrange("p c w -> p (c w)"), sf, ef, OP.subtract)
                P2t = mapp.tile([P, NC5, IMG], BF16, tag="P2")
                M2t = mapp.tile([P, NC5, IMG], BF16, tag="M2")
                nc.scalar.activation(P2t[:], Pt[:], AF.Square)
                nc.scalar.activation(M2t[:], Mt[:], AF.Square)
                return Pt, Mt, P2t, M2t

            def step1(maps, zmaps, k):
                # blur step 1: z[w, j] (transposed, halo layout along w).
                # PSUM bank-paired: [zP|zM] copied by ACT, [zP2|zM2] by DVE.
                Pt, Mt, P2t, M2t = maps
                zPM, z22 = zmaps
                pzPM = psz.tile([P, 2, IMG], F32, tag="pzPM")
                pz22 = psz.tile([P, 2, IMG], F32, tag="pz22")
                for half, src in ((0, Pt), (1, Mt)):
                    for c in range(NC5):
                        rhs, w = band_ap(0, c)
                        nc.tensor.matmul(
                            pzPM[:, half, O[c]:O[c + 1]],
                            src[:, c, R[k]:R[k] + P], rhs,
                            start=(c == 0), stop=(c == NC5 - 1))
                for half, src in ((0, P2t), (1, M2t)):
                    for c in range(NC5):
                        rhs, w = band_ap(0, c)
                        nc.tensor.matmul(
                            pz22[:, half, O[c]:O[c + 1]],
                            src[:, c, R[k]:R[k] + P], rhs,
                            start=(c == 0), stop=(c == NC5 - 1))
                nc.scalar.copy(zPM[:, k, :, :], pzPM[:])
                nc.vector.tensor_copy(z22[:, k, :, :], pz22[:])

            def step2(zmaps, ro, k):
                # blur step 2 (bank-paired [pa|pb], [pu|pv]) + PSUM readout
                zPM, z22 = zmaps
                xy, tuv = ro
                wk = O[k + 1] - O[k]
                pab = ps2.tile([P, 2, IMG], F32, tag="pab")
                puv = ps2.tile([P, 2, IMG], F32, tag="puv")
                bmu, _ = band_ap(1, k)
                bph, _ = band_ap(2, k)
                bnh, _ = band_ap(3, k)
                nc.tensor.matmul(pab[:wk, 0, :], bmu, zPM[:, k, 0, :], start=True, stop=True)
                nc.tensor.matmul(pab[:wk, 1, :], bmu, zPM[:, k, 1, :], start=True, stop=True)
                nc.tensor.matmul(puv[:wk, 0, :], bph, z22[:, k, 0, :], start=True, stop=False)
                nc.tensor.matmul(puv[:wk, 0, :], bph, z22[:, k, 1, :], start=False, stop=True)
                nc.tensor.matmul(puv[:wk, 1, :], bph, z22[:, k, 0, :], start=True, stop=False)
                nc.tensor.matmul(puv[:wk, 1, :], bnh, z22[:, k, 1, :], start=False, stop=True)
                # x,y = pa^2,pb^2 ; tu,tv = pu,pv + C2  (FD=1024 each)
                nc.scalar.activation(xy[:wk, k, :, :], pab[:wk, :, :], AF.Square)
                nc.scalar.activation(tuv[:wk, k, :, :], puv[:wk, :, :], AF.Identity, bias=c2c[:wk, :])

            def chain(ro, b, pi):
                # pointwise chain, TT-heavy (bf16 2x), per chunk-pair so it
                # pipelines with step-2. Garbage partition rows (wk..128)
                # never reach the reduction.
                xy, tuv = ro
                c0, w = PAIRS[pi]
                wk = O[c0 + 1] - O[c0]
                xs = xy[:, c0:c0 + w, 0, :]
                ys = xy[:, c0:c0 + w, 1, :]
                tus = tuv[:, c0:c0 + w, 0, :]
                tvs = tuv[:, c0:c0 + w, 1, :]
                w1 = chp.tile([P, 2, IMG], BF16, tag="w1")
                w2 = chp.tile([P, 2, IMG], BF16, tag="w2")
                nc.vector.tensor_tensor(w1[:, :w, :], xs, ys, OP.subtract)
                nc.vector.tensor_tensor(w2[:, :w, :], xs, ys, OP.add)
                ga = chp.tile([P, 2, IMG], BF16, tag="ga")
                de = chp.tile([P, 2, IMG], BF16, tag="de")
                nc.vector.tensor_tensor(ga[:, :w, :], tvs, w1[:, :w, :], OP.subtract)
                nc.vector.tensor_tensor(de[:, :w, :], tus, w2[:, :w, :], OP.subtract)
                # num = (w1+C1)*gamma via TS(4x)+TT(2x) — cheaper than STT 1x
                al = chp.tile([P, 2, IMG], BF16, tag="nu", name="alt")
                nc.vector.tensor_scalar_add(al[:, :w, :], w1[:, :w, :], C1)
                nu = chp.tile([P, 2, IMG], BF16, tag="nu", name="nut")
                dn = chp.tile([P, 2, IMG], F32, tag="dn")
                nc.vector.tensor_tensor(nu[:, :w, :], al[:, :w, :], ga[:, :w, :], OP.mult)
                nc.vector.scalar_tensor_tensor(
                    dn[:, :w, :], w2[:, :w, :], C1, de[:, :w, :], OP.add, OP.mult)
                rc = chp.tile([P, 2, IMG], F32, tag="rc")
                nc.vector.reciprocal_approx_fast(rc[:, :w, :], dn[:, :w, :])
                # fused ssim = num*rc with accumulation
                jk = chp.tile([P, 2, IMG], BF16, tag="ga")
                nc.vector.scalar_tensor_tensor(
                    jk[:wk, :w, :], nu[:wk, :w, :], 1.0,
                    rc[:wk, :w, :], OP.mult, OP.mult,
                    accum_out=partials[:wk, b * 3 + pi: b * 3 + pi + 1],
                )

            # Per-image streaming; k-interleaved emission so engine priority
            # order matches the pipeline (copy(k), readout(k), copy(k+1)...).
            for b in range(PER_CORE):
                maps = load_and_premaps(b)
                zm = (zp.tile([P, NC5, 2, IMG], BF16, tag="zPM", name="zPMt"),
                      zp.tile([P, NC5, 2, IMG], BF16, tag="z22", name="z22t"))
                ro = (rop.tile([P, NC5, 2, IMG], BF16, tag="xy", name="xyt"),
                      rop.tile([P, NC5, 2, IMG], BF16, tag="tuv", name="tuvt"))
                for k in range(NC5):
                    step1(maps, zm, k)
                    step2(zm, ro, k)
                    # chain pair p is ready once chunks c0..c0+w done
                    for pi, (c0, w) in enumerate(PAIRS):
                        if c0 + w - 1 == k:
                            chain(ro, b, pi)

            final = accp.tile([P, 1], F32)
            nc.vector.tensor_reduce(final[:], partials[:], mybir.AxisListType.X, OP.add)
            nc.sync.dma_start(out_d[:], final[:])

    nc.compile()
    _CACHE["nc"] = nc
    return nc


def kernel(seg: np.ndarray, edge: np.ndarray) -> np.ndarray:
    nc = _build()
    seg = np.ascontiguousarray(seg, dtype=np.float32).reshape(N_CORES, PER_CORE, IMG, IMG)
    edge = np.ascontiguousarray(edge, dtype=np.float32).reshape(N_CORES, PER_CORE, IMG, IMG)
    in_maps = [{"seg": seg[c], "edge": edge[c]} for c in range(N_CORES)]
    res = run_bass_kernel_spmd(nc, in_maps, list(range(N_CORES)))
    total = 0.0
    for c in range(N_CORES):
        total += float(res.results[c]["out"].astype(np.float64).sum())
    mssim = total / (32.0 * IMG * IMG)
    return np.float32(1.0 - (1.0 + mssim) / 2.0)
